# revision 17
# baseline (speedup 1.0000x reference)
"""Trainium2 Bass kernel for nn_AttentionEvaluatorModel (8-core SPMD, data-parallel over batch).

Math (reference):
    qm  = (query @ Wq1.T + bq1) @ Wq2.T + bq2                     (B, Q, E)
    fm  = (features @ Wf1.T + bf1) @ Wf2.T + bf2                  (B, F, E)
    wts = sigmoid(qm @ fm.T) * (ftw * mask)                       (B, Q, F)
    out = cls(wts @ values)                                       (B, Q, L)

Key restructure: the two mapper layers have no nonlinearity and fm is only used
inside qm @ fm.T, so
    qm @ fm.T = (qm @ Wf2 @ Wf1) @ features.T + (qm @ (Wf2 @ bf1 + bf2)) 1^T
which removes the (B, F, FS)x(FS, H)x(H, E) feature-mapper matmuls entirely
(~10x FLOP reduction) and makes the kernel memory-bound on streaming
features+values. Each of the 8 cores handles B/8 = 2 batches end-to-end; there
is no inter-core communication.

On-chip dataflow per core (128 query tokens = 2 batches x 64):
    q-chain (fp32/f32r): qm, qmw = (qm@Wf2)@Wf1, s = z@bf1 + qm@bf2
    stream 64 feature chunks of [128, FS]:
        cast to bf16 -> PE-transpose -> logitsT[chunk] = feats @ qmwT (bf16 mm)
        sigmoid (ACT) * gate (DVE) -> wtsT [128F, 64Q] fp32
        pooled[b] += wtsT.T @ values_chunk   (f32r matmul, 1 cyc/row)
    cls (fp32/f32r) -> out (128 tok, 128 L)
"""

import numpy as np
from contextlib import ExitStack

from concourse import bass, bacc, tile, mybir
from concourse.bass_utils import run_bass_kernel_spmd
from concourse.masks import make_identity

P = 128
N_CORES = 8
B, Q, F, E = 16, 64, 4096, 512
QS, FS, H, CH, L = 512, 512, 1024, 1024, 128
BPC = B // N_CORES          # batches per core (2)
TOK = BPC * Q               # tokens per core (128)
FCH = F // P                # feature chunks per batch (32)

f32 = mybir.dt.float32
f32r = mybir.dt.float32r
bf16 = mybir.dt.bfloat16
u8 = mybir.dt.uint8

_GRAPH_CACHE = {}


def _r(ap):
    return ap.bitcast(f32r)


def _build(nzb: bool):
    """Build the SPMD single-core graph. nzb: whether bias vectors are nonzero."""
    nc = bacc.Bacc("TRN2", target_bir_lowering=False, debug=False,
                   num_devices=N_CORES)

    feat_d = nc.dram_tensor("features", (BPC, F, FS), f32, kind="ExternalInput").ap()
    vals_d = nc.dram_tensor("values", (BPC, F, E), f32, kind="ExternalInput").ap()
    q_d = nc.dram_tensor("query", (TOK, QS), f32, kind="ExternalInput").ap()
    ftw_d = nc.dram_tensor("ftw", (BPC * FCH, P), f32, kind="ExternalInput").ap()
    msk_d = nc.dram_tensor("mask", (BPC * FCH, P), u8, kind="ExternalInput").ap()
    wq1t_d = nc.dram_tensor("Wq1T", (QS, H), f32, kind="ExternalInput").ap()
    wq2t_d = nc.dram_tensor("Wq2T", (H, E), f32, kind="ExternalInput").ap()
    wf2_d = nc.dram_tensor("Wf2", (E, H), f32, kind="ExternalInput").ap()
    wf1_d = nc.dram_tensor("Wf1", (H, FS), f32, kind="ExternalInput").ap()
    wc1t_d = nc.dram_tensor("Wc1T", (E, CH), f32, kind="ExternalInput").ap()
    wc2t_d = nc.dram_tensor("Wc2T", (CH, L), f32, kind="ExternalInput").ap()
    if nzb:
        bq1_d = nc.dram_tensor("bq1", (1, H), f32, kind="ExternalInput").ap()
        bq2_d = nc.dram_tensor("bq2", (1, E), f32, kind="ExternalInput").ap()
        bf1_d = nc.dram_tensor("bf1", (H // P, P), f32, kind="ExternalInput").ap()
        bf2_d = nc.dram_tensor("bf2", (E // P, P), f32, kind="ExternalInput").ap()
        bc1_d = nc.dram_tensor("bc1", (1, CH), f32, kind="ExternalInput").ap()
        bc2_d = nc.dram_tensor("bc2", (1, L), f32, kind="ExternalInput").ap()
    out_d = nc.dram_tensor("out", (TOK, L), f32, kind="ExternalOutput").ap()

    with tile.TileContext(nc) as tc, ExitStack() as ctx:
        const = ctx.enter_context(tc.tile_pool(name="const", bufs=1))
        wpool = ctx.enter_context(tc.tile_pool(name="wpool", bufs=1))
        qact = ctx.enter_context(tc.tile_pool(name="qact", bufs=1))
        featp = ctx.enter_context(tc.tile_pool(name="featp", bufs=5))
        featbfp = ctx.enter_context(tc.tile_pool(name="featbfp", bufs=5))
        ftp = ctx.enter_context(tc.tile_pool(name="ftp", bufs=24))
        valp = ctx.enter_context(tc.tile_pool(name="valp", bufs=8))
        wtsp = ctx.enter_context(tc.tile_pool(name="wtsp", bufs=16))
        qcls_ps = ctx.enter_context(tc.tile_pool(name="qcls_ps", bufs=1, space="PSUM"))
        ft_ps = ctx.enter_context(tc.tile_pool(name="ft_ps", bufs=2, space="PSUM"))
        lg_ps = ctx.enter_context(tc.tile_pool(name="lg_ps", bufs=2, space="PSUM"))
        pool_ps = ctx.enter_context(tc.tile_pool(name="pool_ps", bufs=2, space="PSUM"))

        ident_f = const.tile([P, P], f32)
        make_identity(nc, ident_f)
        ident_bf = const.tile([P, P], bf16)
        make_identity(nc, ident_bf)

        # ---- gate = ftw * mask, laid out [128 (F within chunk), 64 (chunk)] ---
        ftw_n = wpool.tile([BPC * FCH, P], f32)
        nc.sync.dma_start(ftw_n[:], ftw_d[:])
        msk_n = wpool.tile([BPC * FCH, P], u8)
        nc.sync.dma_start(msk_n[:], msk_d[:])
        mskf_n = wpool.tile([BPC * FCH, P], f32)
        nc.vector.tensor_copy(mskf_n[:], msk_n[:])
        gate_n = wpool.tile([BPC * FCH, P], f32)
        nc.vector.tensor_mul(gate_n[:], ftw_n[:], mskf_n[:])
        gate_ps = qcls_ps.tile([P, BPC * FCH], f32, tag="qcls")
        nc.tensor.transpose(gate_ps[:], gate_n[:], ident_f[:BPC * FCH, :BPC * FCH])
        gate_t = wpool.tile([P, BPC * FCH], f32)
        nc.vector.tensor_copy(gate_t[:], gate_ps[:])

        q_sb = qact.tile([P, QS], f32)
        nc.sync.dma_start(q_sb[:], q_d[:])

        # ---- resident weights -------------------------------------------------
        def load_w(name, dram, kin, dout):
            t = wpool.tile([P, kin, dout], f32r, name=name)
            for k in range(kin):
                nc.sync.dma_start(t[:, k, :], dram[k * P:(k + 1) * P, :].bitcast(f32r))
            return t

        wq1t = load_w("wq1t", wq1t_d, QS // P, H)
        wq2t = load_w("wq2t", wq2t_d, H // P, E)
        wf2 = load_w("wf2", wf2_d, E // P, H)
        wf1 = load_w("wf1", wf1_d, H // P, FS)

        # ---- helpers ----------------------------------------------------------
        def transpose_nat(src, nk, name, tag=None):
            """[P, nk*P] f32 SBUF -> [P, nk, P] f32 SBUF (PE transpose per 128-block)."""
            ps = qcls_ps.tile([P, nk, P], f32, tag="qcls", name=name + "_ps")
            for k in range(nk):
                nc.tensor.transpose(ps[:, k, :], src[:, k * P:(k + 1) * P], ident_f[:])
            t = qact.tile([P, nk, P], f32r, name=name, tag=tag or name)
            nc.vector.tensor_copy(t[:], ps[:])
            return t

        def linear_nat(actT, nk, w, dout, bias_row=None, relu=False, name="lin",
                       tag=None):
            """natural-domain linear: out[tok, dout] = actT.T @ w (+ bias) on f32r."""
            ps = qcls_ps.tile([P, dout], f32, tag="qcls", name=name + "_ps")
            for j in range(dout // 512):
                sl = slice(j * 512, (j + 1) * 512)
                for k in range(nk):
                    nc.tensor.matmul(ps[:, sl], actT[:, k, :], w[:, k, sl],
                                     start=(k == 0), stop=(k == nk - 1 and bias_row is None))
                if bias_row is not None:
                    nc.tensor.matmul(ps[:, sl], ones_f[:1, :], bias_row[:1, sl],
                                     start=False, stop=True)
            out = qact.tile([P, dout], f32, name=name, tag=tag or name)
            if relu:
                nc.scalar.activation(out[:], ps[:], mybir.ActivationFunctionType.Relu)
            else:
                nc.vector.tensor_copy(out[:], ps[:])
            return out

        if nzb:
            ones_f = const.tile([1, P], f32)
            nc.vector.memset(ones_f[:], 1.0)
            bq1_row = wpool.tile([1, H], f32)
            nc.sync.dma_start(bq1_row[:], bq1_d[:])
            bq2_row = wpool.tile([1, E], f32)
            nc.sync.dma_start(bq2_row[:], bq2_d[:])
            bc1_row = wpool.tile([1, CH], f32)
            nc.sync.dma_start(bc1_row[:], bc1_d[:])
            bc2_row = wpool.tile([1, L], f32)
            nc.sync.dma_start(bc2_row[:], bc2_d[:])
            # bf1 as [P, 8] / bf2 as [P, 4] column tiles (via natural load + transpose)
            bf1_n = wpool.tile([H // P, P], f32)
            nc.sync.dma_start(bf1_n[:], bf1_d[:])
            bf1_ps = qcls_ps.tile([P, H // P], f32, tag="qcls")
            nc.tensor.transpose(bf1_ps[:], bf1_n[:], ident_f[:H // P, :H // P])
            bf1_c = wpool.tile([P, H // P], f32r)
            nc.scalar.copy(bf1_c[:], bf1_ps[:])
            bf2_n = wpool.tile([E // P, P], f32)
            nc.sync.dma_start(bf2_n[:], bf2_d[:])
            bf2_ps = qcls_ps.tile([P, E // P], f32, tag="qcls")
            nc.tensor.transpose(bf2_ps[:], bf2_n[:], ident_f[:E // P, :E // P])
            bf2_c = wpool.tile([P, E // P], f32r)
            nc.scalar.copy(bf2_c[:], bf2_ps[:])

        # ---- q-chain ----------------------------------------------------------
        qT = transpose_nat(q_sb, QS // P, "qT", tag="qT")
        h1 = linear_nat(qT, QS // P, wq1t, H, bias_row=bq1_row if nzb else None, name="h1", tag="qnat")
        h1T = transpose_nat(h1, H // P, "h1T", tag="qnatT")
        qm = linear_nat(h1T, H // P, wq2t, E, bias_row=bq2_row if nzb else None, name="qm", tag="qnat2")
        qmT = transpose_nat(qm, E // P, "qmT", tag="qmT")
        z = linear_nat(qmT, E // P, wf2, H, name="z", tag="qnat")
        zT = transpose_nat(z, H // P, "zT", tag="qnatT")
        qmw = linear_nat(zT, H // P, wf1, FS, name="qmw", tag="qnat2")
        # qmwT in bf16 for the logits matmuls
        qmwT_ps = qcls_ps.tile([P, FS // P, P], f32, tag="qcls")
        for k in range(FS // P):
            nc.tensor.transpose(qmwT_ps[:, k, :], qmw[:, k * P:(k + 1) * P], ident_f[:])
        qmwT_bf = wpool.tile([P, FS // P, P], bf16)
        nc.vector.tensor_copy(qmwT_bf[:], qmwT_ps[:])

        if nzb:
            # s[tok] = z @ bf1 + qm @ bf2   (row layout [1, TOK])
            s_ps = qcls_ps.tile([1, P], f32, tag="qcls")
            for k in range(H // P):
                nc.tensor.matmul(s_ps[:], bf1_c[:, k:k + 1], zT[:, k, :],
                                 start=(k == 0), stop=False)
            for k in range(E // P):
                nc.tensor.matmul(s_ps[:], bf2_c[:, k:k + 1], qmT[:, k, :],
                                 start=False, stop=(k == E // P - 1))
            s_row = wpool.tile([1, P], f32)
            nc.scalar.copy(s_row[:], s_ps[:])
            ones_f1 = ones_f

        # ---- feature/value stream --------------------------------------------
        # Software-pipelined with a skew: stage 1 (DMA + cast + PE transpose +
        # copyback) has no dependency on qmw and keeps every engine's static
        # order free of cross-stage head-of-line blocking; stage 2 (logits,
        # sigmoid, gate, pooled) trails SKEW chunks behind.
        SKEW = 4
        chunks = [(b, c) for b in range(BPC) for c in range(FCH)]
        st1 = {}
        pooled = None
        wc1t = wc2t = None
        for i in range(len(chunks) + SKEW):
            if i < len(chunks):
                b, c = chunks[i]
                rows = slice(c * P, (c + 1) * P)
                vchunk = valp.tile([P, E], f32r, tag="vchunk")
                nc.gpsimd.dma_start(vchunk[:], vals_d[b, rows, :].bitcast(f32r))
                fchunk = featp.tile([P, FS], f32, tag="fchunk")
                nc.sync.dma_start(fchunk[:], feat_d[b, rows, :])
                fbf = featbfp.tile([P, FS], bf16, tag="fbf")
                nc.vector.tensor_copy(fbf[:], fchunk[:])
                ftps = ft_ps.tile([P, FS // P, P], bf16, tag="ftps")
                for k in range(FS // P):
                    nc.tensor.transpose(ftps[:, k, :], fbf[:, k * P:(k + 1) * P],
                                        ident_bf[:])
                ft_sb = ftp.tile([P, FS // P, P], bf16, tag="ft_sb")
                nc.scalar.copy(ft_sb[:], ftps[:])
                st1[i] = (ft_sb, vchunk)
            if i >= SKEW:
                b, c = chunks[i - SKEW]
                ft_sb, vchunk = st1.pop(i - SKEW)
                if c == 0:
                    pooled = pool_ps.tile([Q, E], f32, tag="pooled",
                                          name=f"pooled{b}")
                lg = lg_ps.tile([P, Q], f32, tag="lg")
                for k in range(FS // P):
                    nc.tensor.matmul(lg[:], ft_sb[:, k, :],
                                     qmwT_bf[:, k, b * Q:(b + 1) * Q],
                                     start=(k == 0),
                                     stop=(k == FS // P - 1 and not nzb))
                if nzb:
                    nc.tensor.matmul(lg[:], ones_f1[:1, :],
                                     s_row[:1, b * Q:(b + 1) * Q],
                                     start=False, stop=True)
                wts = wtsp.tile([P, Q], f32, tag="wts")
                nc.scalar.activation(wts[:], lg[:],
                                     mybir.ActivationFunctionType.Sigmoid)
                wtsg = wtsp.tile([P, Q], f32r, tag="wtsg")
                nc.vector.tensor_scalar_mul(wtsg[:], wts[:],
                                            gate_t[:, b * FCH + c:b * FCH + c + 1])
                nc.tensor.matmul(pooled[:], wtsg[:], vchunk[:],
                                 start=(c == 0), stop=(c == FCH - 1))
                if c == FCH - 1:
                    # ---- per-batch cls head ---------------------------------
                    if wc1t is None:
                        wc1t = load_w("wc1t", wc1t_d, E // P, CH)
                        wc2t = load_w("wc2t", wc2t_d, CH // P, L)
                    psb = qact.tile([Q, E], f32, name=f"pooled_sb{b}", tag="pooled_sb")
                    nc.vector.tensor_copy(psb[:], pooled[:])
                    pooledT_ps = qcls_ps.tile([P, E // P, Q], f32, tag="qcls",
                                              name=f"pooledT_ps{b}")
                    for k in range(E // P):
                        nc.tensor.transpose(pooledT_ps[:, k, :],
                                            psb[:, k * P:(k + 1) * P],
                                            ident_f[:Q, :Q])
                    pooledT = qact.tile([P, E // P, Q], f32r, name=f"pooledT{b}", tag="pooledT")
                    nc.vector.tensor_copy(pooledT[:], pooledT_ps[:])
                    h_ps = qcls_ps.tile([Q, CH], f32, tag="qcls", name=f"h_ps{b}")
                    for j in range(CH // 512):
                        sl = slice(j * 512, (j + 1) * 512)
                        for k in range(E // P):
                            nc.tensor.matmul(h_ps[:, sl], pooledT[:, k, :],
                                             wc1t[:, k, sl], start=(k == 0),
                                             stop=(k == E // P - 1 and not nzb))
                        if nzb:
                            nc.tensor.matmul(h_ps[:, sl], ones_f[:1, :Q],
                                             bc1_row[:1, sl], start=False, stop=True)
                    h_sb = qact.tile([Q, CH], f32, name=f"h_sb{b}", tag="h_sb")
                    nc.scalar.activation(h_sb[:], h_ps[:],
                                         mybir.ActivationFunctionType.Relu)
                    hT_ps = qcls_ps.tile([P, CH // P, Q], f32, tag="qcls",
                                         name=f"hT_ps{b}")
                    for k in range(CH // P):
                        nc.tensor.transpose(hT_ps[:, k, :],
                                            h_sb[:, k * P:(k + 1) * P],
                                            ident_f[:Q, :Q])
                    hT = qact.tile([P, CH // P, Q], f32r, name=f"hT{b}", tag="hT")
                    nc.vector.tensor_copy(hT[:], hT_ps[:])
                    out_ps = qcls_ps.tile([Q, L], f32, tag="qcls", name=f"out_ps{b}")
                    for k in range(CH // P):
                        nc.tensor.matmul(out_ps[:], hT[:, k, :], wc2t[:, k, :],
                                         start=(k == 0),
                                         stop=(k == CH // P - 1 and not nzb))
                    if nzb:
                        nc.tensor.matmul(out_ps[:], ones_f[:1, :Q], bc2_row[:1, :],
                                         start=False, stop=True)
                    out_sb = qact.tile([Q, L], f32, name=f"out_sb{b}", tag="out_sb")
                    nc.vector.tensor_copy(out_sb[:], out_ps[:])
                    nc.sync.dma_start(out_d[b * Q:(b + 1) * Q, :], out_sb[:])

    nc.compile()
    return nc


def run(inputs, trace=False, tmpdir=None):
    q = np.ascontiguousarray(np.asarray(inputs["query"], dtype=np.float32))
    feats = np.ascontiguousarray(np.asarray(inputs["features"], dtype=np.float32))
    vals = np.ascontiguousarray(np.asarray(inputs["values"], dtype=np.float32))
    ftw = np.ascontiguousarray(np.asarray(inputs["feature_time_weights"], dtype=np.float32))
    mask = np.asarray(inputs["attention_mask"])
    biases = {k: np.ascontiguousarray(np.asarray(inputs[k], dtype=np.float32))
              for k in ("bq1", "bq2", "bf1", "bf2", "bc1", "bc2")}
    biases = {k: (v.reshape(-1, 128) if k in ("bf1", "bf2") else v.reshape(1, -1))
              for k, v in biases.items()}
    nzb = any(np.any(v) for v in biases.values())

    if nzb not in _GRAPH_CACHE:
        _GRAPH_CACHE[nzb] = _build(nzb)
    nc = _GRAPH_CACHE[nzb]

    shared = {
        "Wq1T": np.ascontiguousarray(np.asarray(inputs["Wq1"], dtype=np.float32).T),
        "Wq2T": np.ascontiguousarray(np.asarray(inputs["Wq2"], dtype=np.float32).T),
        "Wf2": np.ascontiguousarray(np.asarray(inputs["Wf2"], dtype=np.float32)),
        "Wf1": np.ascontiguousarray(np.asarray(inputs["Wf1"], dtype=np.float32)),
        "Wc1T": np.ascontiguousarray(np.asarray(inputs["Wc1"], dtype=np.float32).T),
        "Wc2T": np.ascontiguousarray(np.asarray(inputs["Wc2"], dtype=np.float32).T),
    }
    if nzb:
        shared.update(biases)

    in_maps = []
    for cidx in range(N_CORES):
        bs = slice(cidx * BPC, (cidx + 1) * BPC)
        in_maps.append(dict(
            shared,
            features=np.ascontiguousarray(feats[bs]),
            values=np.ascontiguousarray(vals[bs]),
            query=np.ascontiguousarray(q[bs].reshape(TOK, QS)),
            ftw=np.ascontiguousarray(ftw[bs].reshape(BPC * FCH, P)),
            mask=np.ascontiguousarray(mask[bs].reshape(BPC * FCH, P)).astype(np.uint8),
        ))

    res = run_bass_kernel_spmd(nc, in_maps, core_ids=list(range(N_CORES)),
                               trace=trace, tmpdir=tmpdir)
    out = np.concatenate(
        [res.results[i]["out"].reshape(BPC, Q, L) for i in range(N_CORES)], axis=0)
    return out, res


def kernel(**inputs) -> np.ndarray:
    out, _ = run(inputs, trace=False)
    return out

